# revision 6
# baseline (speedup 1.0000x reference)
"""Multi-head attention (b=16, l=1025, d=768, H=12) on 8 TRN2 NeuronCores.

Sharding: data-parallel over batch — 2 batch elements per core, no
collectives.

Per-core kernel (per batch element), all layouts transposed so the
sequence dim is the matmul free dim:
  1. QK^T = (Wqk stationary) @ X^T            -> [1536, L]  (bf16)
  2. V    = (X^T blocks stationary) @ Wv      -> [L, 768] stored per-head
     as [L, 12*(64+1)] with a ones column per head (gives softmax sums).
  3. Per head pair: S^T[jblk, i] = K_h^T.T @ Q_h^T (K=64, row-group packed)
     P^T = exp(S^T/8) on ACT (no max subtraction: |scores| <~ 2)
     O_aug^T[65, i] += V_aug[jblk].T @ P^T[jblk]   (row 64 = softmax denom)
     O^T = O_aug^T[0:64] * recip(O_aug^T[64])  (gpsimd bcast + DVE)
  4. Y^T = (Wo stationary) @ O^T + bo         -> [768, L] fp32 -> DRAM

Host side: permute Wqkv from interleaved-head to head-contiguous order,
transpose inputs/outputs, cast to bf16.
"""

import numpy as np
import ml_dtypes

import concourse.bass as bass
import concourse.bacc as bacc
import concourse.mybir as mybir
import concourse.tile as tile
from concourse.bass_utils import run_bass_kernel_spmd

N_CORES = 8
B = 16
L = 1025
D = 768
H = 12
DH = 64  # head dim
BPC = B // N_CORES  # batch elements per core
KT = D // 128  # 6 contraction tiles
JT = (L + 127) // 128  # 9 sequence tiles (last has 1 row)
SCALE = 1.0 / np.sqrt(DH)

BF16 = mybir.dt.bfloat16
F32 = mybir.dt.float32

# i-chunks (free-dim chunks over queries): PSUM bank limit is 512 fp32
I_CHUNKS = [(0, 512), (512, 512), (1024, 1)]
# chunks over the 768-wide V projection output
V_CHUNKS = [(0, 512), (512, 256)]

_CACHE = {}


def _build():
    nc = bacc.Bacc("TRN2", target_bir_lowering=False, debug=False,
                   num_devices=N_CORES)

    # DRAM I/O (per core)
    xT = nc.dram_tensor("xT", [BPC, D, L], BF16, kind="ExternalInput")
    w_qk = nc.dram_tensor("w_qk", [D, 2 * D], BF16, kind="ExternalInput")
    w_v = nc.dram_tensor("w_v", [D, D], BF16, kind="ExternalInput")
    w_o = nc.dram_tensor("w_o", [D, D], BF16, kind="ExternalInput")
    b_qk = nc.dram_tensor("b_qk", [2 * D, 1], F32, kind="ExternalInput")
    b_v = nc.dram_tensor("b_v", [1, D], F32, kind="ExternalInput")
    b_o = nc.dram_tensor("b_o", [D, 1], F32, kind="ExternalInput")
    yT = nc.dram_tensor("yT", [BPC, D, L], F32, kind="ExternalOutput")

    with tile.TileContext(nc) as tc:
        _emit(nc, tc, xT, w_qk, w_v, w_o, b_qk, b_v, b_o, yT)
    nc.compile()
    return nc


def _emit(nc, tc, xT, w_qk, w_v, w_o, b_qk, b_v, b_o, yT):
    import contextlib
    ctx = contextlib.ExitStack()
    with ctx:
        consts = ctx.enter_context(tc.tile_pool(name="consts", bufs=1))
        xpool = ctx.enter_context(tc.tile_pool(name="xpool", bufs=1))
        qkpool = ctx.enter_context(tc.tile_pool(name="qkpool", bufs=1))
        vpool = ctx.enter_context(tc.tile_pool(name="vpool", bufs=1))
        otpool = ctx.enter_context(tc.tile_pool(name="otpool", bufs=1))
        ytpool = ctx.enter_context(tc.tile_pool(name="ytpool", bufs=2))
        ptpool = ctx.enter_context(tc.tile_pool(name="ptpool", bufs=6))
        smpool = ctx.enter_context(tc.tile_pool(name="smpool", bufs=4))
        pspool = ctx.enter_context(
            tc.tile_pool(name="pspool", bufs=2, space="PSUM"))
        spool = ctx.enter_context(
            tc.tile_pool(name="spool", bufs=4, space="PSUM"))
        acpool = ctx.enter_context(
            tc.tile_pool(name="acpool", bufs=1, space="PSUM"))

        # ---- weights / biases (loaded once) ----
        wqk_t = [consts.tile([128, 2 * D], BF16, tag=f"wqk{k}", name=f"wqk{k}")
                 for k in range(KT)]
        for k in range(KT):
            nc.sync.dma_start(out=wqk_t[k][:], in_=w_qk[k * 128:(k + 1) * 128, :])
        wv_t = [consts.tile([128, D], BF16, tag=f"wv{k}", name=f"wv{k}") for k in range(KT)]
        for k in range(KT):
            nc.sync.dma_start(out=wv_t[k][:], in_=w_v[k * 128:(k + 1) * 128, :])
        wo_t = [consts.tile([128, D], BF16, tag=f"wo{k}", name=f"wo{k}") for k in range(KT)]
        for k in range(KT):
            nc.sync.dma_start(out=wo_t[k][:], in_=w_o[k * 128:(k + 1) * 128, :])

        # per-partition bias columns for QK^T layout: [128, 1] per m-tile
        bqk_t = [consts.tile([128, 1], F32, tag=f"bqk{m}", name=f"bqk{m}") for m in range(2 * KT)]
        for m in range(2 * KT):
            nc.sync.dma_start(out=bqk_t[m][:],
                              in_=b_qk[m * 128:(m + 1) * 128, :])
        bo_t = [consts.tile([128, 1], F32, tag=f"bo{m}", name=f"bo{m}") for m in range(KT)]
        for m in range(KT):
            nc.sync.dma_start(out=bo_t[m][:],
                              in_=b_o[m * 128:(m + 1) * 128, :])
        # bv broadcast to all 128 partitions: [128, 768]
        bv_bc = consts.tile([128, D], F32, tag="bvbc")
        bva = b_v[:]
        bv_src = bass.AP(tensor=bva.tensor, offset=bva.offset,
                         ap=[[0, 128], list(bva.ap[1])])
        nc.sync.dma_start(out=bv_bc[:], in_=bv_src)

        for e in range(BPC):
            # ---- load X^T (bf16) ----
            xt = [xpool.tile([128, L], BF16, tag=f"xt{k}", name=f"xt{k}") for k in range(KT)]
            for k in range(KT):
                nc.sync.dma_start(out=xt[k][:],
                                  in_=xT[e, k * 128:(k + 1) * 128, :])

            # ---- QK^T projection: [1536, L] ----
            qkT = [qkpool.tile([128, L], BF16, tag=f"qkT{m}", name=f"qkT{m}")
                   for m in range(2 * KT)]
            for m in range(2 * KT):
                for (i0, ilen) in I_CHUNKS:
                    ps = pspool.tile([128, 512], F32, tag="proj_ps")
                    for k in range(KT):
                        nc.tensor.matmul(
                            ps[:, :ilen],
                            wqk_t[k][:, m * 128:(m + 1) * 128],
                            xt[k][:, i0:i0 + ilen],
                            start=(k == 0), stop=(k == KT - 1))
                    nc.vector.tensor_scalar_add(
                        qkT[m][:, i0:i0 + ilen], ps[:, :ilen], bqk_t[m][:])

            # ---- V projection: per j-tile [jlen, 12*65] with ones cols ----
            vt = [vpool.tile([128, H * (DH + 1)], BF16, tag=f"vt{j}", name=f"vt{j}")
                  for j in range(JT)]
            for j in range(JT):
                jlen = min(128, L - j * 128)
                ones_ap = bass.AP(
                    tensor=vt[j].tensor, offset=vt[j].offset + DH,
                    ap=[vt[j].ap[0], [DH + 1, H], [1, 1]])
                nc.vector.memset(ones_ap, 1.0)
                for (c0, clen) in V_CHUNKS:
                    ps = pspool.tile([128, 512], F32, tag="proj_ps")
                    for k in range(KT):
                        nc.tensor.matmul(
                            ps[:jlen, :clen],
                            xt[k][:, j * 128:j * 128 + jlen],
                            wv_t[k][:, c0:c0 + clen],
                            start=(k == 0), stop=(k == KT - 1))
                    # write V into strided per-head slots, adding bias
                    nheads = clen // DH
                    h0 = c0 // DH
                    dst = bass.AP(
                        tensor=vt[j].tensor,
                        offset=vt[j].offset + h0 * (DH + 1),
                        ap=[[vt[j].ap[0][0], jlen], [DH + 1, nheads], [1, DH]])
                    src = bass.AP(
                        tensor=ps.tensor, offset=ps.offset,
                        ap=[[ps.ap[0][0], jlen], [DH, nheads], [1, DH]])
                    bia = bass.AP(
                        tensor=bv_bc.tensor, offset=bv_bc.offset + c0,
                        ap=[[bv_bc.ap[0][0], jlen], [DH, nheads], [1, DH]])
                    nc.vector.tensor_tensor(
                        out=dst, in0=src, in1=bia, op=mybir.AluOpType.add)

            # ---- attention, head pairs ----
            oT = [otpool.tile([128, L], BF16, tag=f"oT{g}", name=f"oT{g}") for g in range(KT)]
            for g in range(KT):  # pair index; heads 2g, 2g+1
                for (i0, ilen) in I_CHUNKS:
                    oacc = [acpool.tile([128, 512], F32, tag=f"oacc{u}", name=f"oacc{u}")
                            for u in range(2)]
                    for j in range(JT):
                        jlen = min(128, L - j * 128)
                        for u in range(2):  # head 2g+u at partitions u*64
                            h = 2 * g + u
                            # S^T[j-block, i-chunk] = K_h^T.T @ Q_h^T
                            sps = spool.tile([128, 512], F32, tag="s_ps")
                            nc.tensor.matmul(
                                sps[:jlen, :ilen],
                                qkT[KT + g][u * 64:(u + 1) * 64,
                                            j * 128:j * 128 + jlen],
                                qkT[g][u * 64:(u + 1) * 64, i0:i0 + ilen],
                                start=True, stop=True)
                            # P^T = exp(S^T * scale)  (bf16)
                            pt = ptpool.tile([128, 512], BF16, tag="pt")
                            nc.scalar.activation(
                                pt[:jlen, :ilen], sps[:jlen, :ilen],
                                mybir.ActivationFunctionType.Exp,
                                bias=0.0, scale=float(SCALE))
                            # O_aug^T += V_aug[j].T @ P^T
                            nc.tensor.matmul(
                                oacc[u][:DH + 1, :ilen],
                                vt[j][:jlen, h * (DH + 1):(h + 1) * (DH + 1)],
                                pt[:jlen, :ilen],
                                start=(j == 0), stop=(j == JT - 1))
                    for u in range(2):
                        # normalize: O^T = O_aug^T[0:64] / O_aug^T[64]
                        rec1 = smpool.tile([1, 512], F32, tag="rec1")
                        nc.vector.reciprocal(
                            rec1[:1, :ilen], oacc[u][DH:DH + 1, :ilen])
                        rec = smpool.tile([128, 512], F32, tag="rec")
                        nc.gpsimd.partition_broadcast(
                            rec[:DH, :ilen], rec1[:1, :ilen])
                        nc.vector.tensor_tensor(
                            out=oT[g][u * 64:(u + 1) * 64, i0:i0 + ilen],
                            in0=oacc[u][:DH, :ilen],
                            in1=rec[:DH, :ilen],
                            op=mybir.AluOpType.mult)

            # ---- output projection: Y^T = Wo.T @ O^T + bo ----
            for m in range(KT):
                yt = ytpool.tile([128, L], F32, tag=f"yt{m}")
                for (i0, ilen) in I_CHUNKS:
                    ps = pspool.tile([128, 512], F32, tag="proj_ps")
                    for k in range(KT):
                        nc.tensor.matmul(
                            ps[:, :ilen],
                            wo_t[k][:, m * 128:(m + 1) * 128],
                            oT[k][:, i0:i0 + ilen],
                            start=(k == 0), stop=(k == KT - 1))
                    nc.vector.tensor_scalar_add(
                        yt[:, i0:i0 + ilen], ps[:, :ilen], bo_t[m][:])
                nc.sync.dma_start(out=yT[e, m * 128:(m + 1) * 128, :],
                                  in_=yt[:])


def _prep_inputs(query, Wqkv, bqkv, Wo, bo):
    # interleaved-head -> head-contiguous channel permutation:
    # col (s*64 + d)*12 + h  ->  (s, h, d)
    Wp = Wqkv.reshape(D, 3, DH, H).transpose(0, 1, 3, 2).reshape(D, 3 * D)
    bp = bqkv.reshape(3, DH, H).transpose(0, 2, 1).reshape(3 * D)
    w_qk = np.ascontiguousarray(Wp[:, :2 * D]).astype(ml_dtypes.bfloat16)
    w_v = np.ascontiguousarray(Wp[:, 2 * D:]).astype(ml_dtypes.bfloat16)
    w_o = np.ascontiguousarray(Wo).astype(ml_dtypes.bfloat16)
    b_qk = np.ascontiguousarray(bp[:2 * D]).astype(np.float32).reshape(2 * D, 1)
    b_v = np.ascontiguousarray(bp[2 * D:]).astype(np.float32).reshape(1, D)
    b_o = np.ascontiguousarray(bo).astype(np.float32).reshape(D, 1)

    in_maps = []
    for c in range(N_CORES):
        xc = query[c * BPC:(c + 1) * BPC]          # [BPC, L, D]
        xT = np.ascontiguousarray(xc.transpose(0, 2, 1)).astype(
            ml_dtypes.bfloat16)                     # [BPC, D, L]
        in_maps.append(dict(xT=xT, w_qk=w_qk, w_v=w_v, w_o=w_o,
                            b_qk=b_qk, b_v=b_v, b_o=b_o))
    return in_maps


def kernel(query, Wqkv, bqkv, Wo, bo):
    query = np.asarray(query, dtype=np.float32)
    Wqkv = np.asarray(Wqkv, dtype=np.float32)
    bqkv = np.asarray(bqkv, dtype=np.float32)
    Wo = np.asarray(Wo, dtype=np.float32)
    bo = np.asarray(bo, dtype=np.float32)

    if "nc" not in _CACHE:
        _CACHE["nc"] = _build()
    nc = _CACHE["nc"]

    in_maps = _prep_inputs(query, Wqkv, bqkv, Wo, bo)
    res = run_bass_kernel_spmd(nc, in_maps, core_ids=list(range(N_CORES)))
    out = np.empty((B, L, D), dtype=np.float32)
    for c in range(N_CORES):
        out[c * BPC:(c + 1) * BPC] = res.results[c]["yT"].transpose(0, 2, 1)
    return out


# revision 9
# speedup vs baseline: 1.1660x; 1.1660x over previous
"""Multi-head attention (b=16, l=1025, d=768, H=12) on 8 TRN2 NeuronCores.

Sharding: data-parallel over batch - 2 batch elements per core, no
collectives.

Per-core kernel (per batch element), layouts transposed so the sequence
dim is the matmul free dim:
  1. QK^T = (Wqk stationary) @ X^T            -> [1536, L]  (bf16)
  2. V    = (X^T blocks stationary) @ Wv      -> [L, 768] stored per-head
     as [L, 12*(64+1)] with a ones column per head (gives softmax sums).
  3. Per head pair g (heads 2g, 2g+1 at partitions 0-63 / 64-127, PE
     row-group packed), per i-chunk c in {[0:512], [512:1024]}:
       S^T[jblk, i] = K_h^T.T @ Q_h^T   (K=64)
       P^T = exp(S^T/8) on ACT, grouped 2 j-tiles per instruction
       O_aug^T[65, i] += V_aug[jblk].T @ P^T[jblk]  (row 64 = denom)
       O^T = O_aug^T[0:64] * recip(O_aug^T[64])
     query col 1024 handled via a [128, 18] psum tile + one exp.
  4. Y^T = (Wo stationary) @ O^T + bo         -> [768, L] fp32 -> DRAM

Projection work of element 1 is interleaved into element 0's attention
pairs (and elem0's output projection into elem1's attention) so the PE
has dense work while ACT drains the exps.
"""

import contextlib

import numpy as np
import ml_dtypes

import concourse.bass as bass
import concourse.bacc as bacc
import concourse.mybir as mybir
import concourse.tile as tile
from concourse.bass_utils import run_bass_kernel_spmd

N_CORES = 8
B = 16
L = 1025
D = 768
H = 12
DH = 64
BPC = B // N_CORES
KT = D // 128   # 6 contraction tiles
JT = (L + 127) // 128  # 9 j-tiles; last has 1 row
SCALE = 1.0 / np.sqrt(DH)

BF16 = mybir.dt.bfloat16
F32 = mybir.dt.float32
EXP = mybir.ActivationFunctionType.Exp
MULT = mybir.AluOpType.mult
ADD = mybir.AluOpType.add

_CACHE = {}


def _build():
    nc = bacc.Bacc("TRN2", target_bir_lowering=False, debug=False,
                   num_devices=N_CORES)
    xT = nc.dram_tensor("xT", [BPC, D, L], BF16, kind="ExternalInput")
    w_qk = nc.dram_tensor("w_qk", [D, 2 * D], BF16, kind="ExternalInput")
    w_v = nc.dram_tensor("w_v", [D, D], BF16, kind="ExternalInput")
    w_o = nc.dram_tensor("w_o", [D, D], BF16, kind="ExternalInput")
    b_qk = nc.dram_tensor("b_qk", [2 * D, 1], F32, kind="ExternalInput")
    b_v = nc.dram_tensor("b_v", [1, D], F32, kind="ExternalInput")
    b_o = nc.dram_tensor("b_o", [D, 1], F32, kind="ExternalInput")
    yT = nc.dram_tensor("yT", [BPC, D, L], F32, kind="ExternalOutput")

    with tile.TileContext(nc) as tc:
        _emit(nc, tc, xT, w_qk, w_v, w_o, b_qk, b_v, b_o, yT)
    nc.compile()
    return nc


def _ap(t, poff, pcount, foff, fdims):
    """AP on tile t at partition offset poff (count pcount), free offset
    foff with free dims [(step, count), ...]."""
    base = t[:]
    pstep = base.ap[0][0]
    return bass.AP(tensor=base.tensor,
                   offset=base.offset + poff * pstep + foff,
                   ap=[[pstep, pcount]] + [list(d) for d in fdims])


def _emit(nc, tc, xT, w_qk, w_v, w_o, b_qk, b_v, b_o, yT):
    ctx = contextlib.ExitStack()
    with ctx:
        consts = ctx.enter_context(tc.tile_pool(name="consts", bufs=1))
        xpool = ctx.enter_context(tc.tile_pool(name="xpool", bufs=1))
        qkpool = ctx.enter_context(tc.tile_pool(name="qkpool", bufs=2))
        vpool = ctx.enter_context(tc.tile_pool(name="vpool", bufs=2))
        otpool = ctx.enter_context(tc.tile_pool(name="otpool", bufs=2))
        ytpool = ctx.enter_context(tc.tile_pool(name="ytpool", bufs=2))
        ptpool = ctx.enter_context(tc.tile_pool(name="ptpool", bufs=3))
        smpool = ctx.enter_context(tc.tile_pool(name="smpool", bufs=2))
        # PSUM: big 2x[128,1024]=4 banks, small 1, acc 2, accsg 1 -> 8
        bigp = ctx.enter_context(tc.tile_pool(name="bigp", bufs=2, space="PSUM"))
        smallp = ctx.enter_context(tc.tile_pool(name="smallp", bufs=1, space="PSUM"))
        accp = ctx.enter_context(tc.tile_pool(name="accp", bufs=2, space="PSUM"))
        sgp = ctx.enter_context(tc.tile_pool(name="sgp", bufs=1, space="PSUM"))

        # ---- constants ----
        wqk_t = [consts.tile([128, 2 * D], BF16, name=f"wqk{k}") for k in range(KT)]
        wv_t = [consts.tile([128, D], BF16, name=f"wv{k}") for k in range(KT)]
        wo_t = [consts.tile([128, D], BF16, name=f"wo{k}") for k in range(KT)]
        for k in range(KT):
            nc.sync.dma_start(out=wqk_t[k][:], in_=w_qk[k * 128:(k + 1) * 128, :])
            nc.sync.dma_start(out=wv_t[k][:], in_=w_v[k * 128:(k + 1) * 128, :])
            nc.sync.dma_start(out=wo_t[k][:], in_=w_o[k * 128:(k + 1) * 128, :])
        bqk_t = [consts.tile([128, 1], F32, name=f"bqk{m}") for m in range(2 * KT)]
        for m in range(2 * KT):
            nc.sync.dma_start(out=bqk_t[m][:], in_=b_qk[m * 128:(m + 1) * 128, :])
        bo_t = [consts.tile([128, 1], F32, name=f"bo{m}") for m in range(KT)]
        for m in range(KT):
            nc.sync.dma_start(out=bo_t[m][:], in_=b_o[m * 128:(m + 1) * 128, :])
        bv_bc = consts.tile([128, D], F32, name="bvbc")
        bva = b_v[:]
        nc.sync.dma_start(out=bv_bc[:], in_=bass.AP(
            tensor=bva.tensor, offset=bva.offset,
            ap=[[0, 128], list(bva.ap[1])]))

        xt = {}
        qkT = {}
        vt = {}
        oT = {}

        def load_x(e):
            xt[e] = [xpool.tile([128, L], BF16, tag=f"xt{k}", name=f"xt{e}_{k}")
                     for k in range(KT)]
            for k in range(KT):
                nc.sync.dma_start(out=xt[e][k][:],
                                  in_=xT[e, k * 128:(k + 1) * 128, :])

        def v_proj(e, jlist):
            """V[j,:] for j-tiles in jlist; layout [jlen, 12*(64+1)]."""
            if e not in vt:
                vt[e] = [vpool.tile([128, H * (DH + 1)], BF16, tag=f"vt{j}",
                                    name=f"vt{e}_{j}") for j in range(JT)]
            for j in jlist:
                jlen = min(128, L - j * 128)
                nc.vector.memset(
                    _ap(vt[e][j], 0, 128, DH, [[DH + 1, H], [1, 1]]), 1.0)
                ps = bigp.tile([128, 1024], F32, tag="big", name=f"vps{e}_{j}")
                for k in range(KT):
                    nc.tensor.matmul(ps[:jlen, 0:512],
                                     xt[e][k][:, j * 128:j * 128 + jlen],
                                     wv_t[k][:, 0:512],
                                     start=(k == 0), stop=(k == KT - 1))
                for k in range(KT):
                    nc.tensor.matmul(ps[:jlen, 512:768],
                                     xt[e][k][:, j * 128:j * 128 + jlen],
                                     wv_t[k][:, 512:768],
                                     start=(k == 0), stop=(k == KT - 1))
                dst = _ap(vt[e][j], 0, jlen, 0, [[DH + 1, H], [1, DH]])
                src = _ap(ps, 0, jlen, 0, [[DH, H], [1, DH]])
                bia = _ap(bv_bc, 0, jlen, 0, [[DH, H], [1, DH]])
                nc.vector.tensor_tensor(out=dst, in0=src, in1=bia, op=ADD)

        def qk_proj(e, mlist):
            """QK^T rows for m-tiles in mlist."""
            if e not in qkT:
                qkT[e] = [qkpool.tile([128, L], BF16, tag=f"qkT{m}",
                                      name=f"qkT{e}_{m}") for m in range(2 * KT)]
            sg = smallp.tile([128, 512], F32, tag="small", name=f"qksg{e}_{mlist[0]}")
            for mi, m in enumerate(mlist):
                ps = bigp.tile([128, 1024], F32, tag="big", name=f"qkps{e}_{m}")
                for k in range(KT):
                    nc.tensor.matmul(ps[:, 0:512],
                                     wqk_t[k][:, m * 128:(m + 1) * 128],
                                     xt[e][k][:, 0:512],
                                     start=(k == 0), stop=(k == KT - 1))
                for k in range(KT):
                    nc.tensor.matmul(ps[:, 512:1024],
                                     wqk_t[k][:, m * 128:(m + 1) * 128],
                                     xt[e][k][:, 512:1024],
                                     start=(k == 0), stop=(k == KT - 1))
                for k in range(KT):
                    nc.tensor.matmul(sg[:, mi:mi + 1],
                                     wqk_t[k][:, m * 128:(m + 1) * 128],
                                     xt[e][k][:, 1024:1025],
                                     start=(k == 0), stop=(k == KT - 1))
                nc.vector.tensor_scalar_add(qkT[e][m][:, 0:1024], ps[:, 0:1024],
                                            bqk_t[m][:])
            for mi, m in enumerate(mlist):
                nc.vector.tensor_scalar_add(qkT[e][m][:, 1024:1025],
                                            sg[:, mi:mi + 1], bqk_t[m][:])

        def attention(e, g):
            """Head pair g: heads 2g (partitions 0-63), 2g+1 (64-127)."""
            if e not in oT:
                oT[e] = [otpool.tile([128, L], BF16, tag=f"oT{t}",
                                     name=f"oT{e}_{t}") for t in range(KT)]
            kt_q, kt_k = qkT[e][g], qkT[e][KT + g]
            accsg = sgp.tile([128, 8], F32, tag="sg", name=f"accsg{e}_{g}")
            for (i0, ilen) in ((0, 512), (512, 512)):
                oacc = [accp.tile([128, 512], F32, tag="acc",
                                  name=f"oacc{e}_{g}_{i0}_{u}") for u in range(2)]
                # j-groups {0,1},{2,3},{4,5},{6,7} per u; j8 u0/u1 share one
                for u in range(2):
                    for jg in range(4):
                        j0 = 2 * jg
                        sps = bigp.tile([128, 1024], F32, tag="big",
                                        name=f"sps{e}_{g}_{i0}_{u}_{jg}")
                        for jj in range(2):
                            j = j0 + jj
                            nc.tensor.matmul(
                                sps[:128, jj * 512:jj * 512 + ilen],
                                kt_k[u * 64:(u + 1) * 64, j * 128:(j + 1) * 128],
                                kt_q[u * 64:(u + 1) * 64, i0:i0 + ilen],
                                start=True, stop=True)
                        pt = ptpool.tile([128, 1024], BF16, tag="pt",
                                         name=f"pt{e}_{g}_{i0}_{u}_{jg}")
                        nc.scalar.activation(pt[:, :], sps[:, :], EXP,
                                             bias=0.0, scale=float(SCALE))
                        for jj in range(2):
                            j = j0 + jj
                            h = 2 * g + u
                            nc.tensor.matmul(
                                oacc[u][:DH + 1, :ilen],
                                vt[e][j][:, h * (DH + 1):(h + 1) * (DH + 1)],
                                pt[:, jj * 512:jj * 512 + ilen],
                                start=(j == 0), stop=False)
                # j8 (jlen=1): u0 in cols 0:512, u1 in cols 512:1024,
                # both at partition 0 so PV lhsT/rhs bases match
                sp8 = bigp.tile([128, 1024], F32, tag="big",
                                name=f"sp8{e}_{g}_{i0}")
                for u in range(2):
                    nc.tensor.matmul(
                        sp8[0:1, u * 512:u * 512 + ilen],
                        kt_k[u * 64:(u + 1) * 64, 1024:1025],
                        kt_q[u * 64:(u + 1) * 64, i0:i0 + ilen],
                        start=True, stop=True)
                pt8 = ptpool.tile([1, 1024], BF16, tag="pt8",
                                  name=f"pt8{e}_{g}_{i0}")
                nc.scalar.activation(pt8[:1, :], sp8[:1, :], EXP,
                                     bias=0.0, scale=float(SCALE))
                for u in range(2):
                    h = 2 * g + u
                    nc.tensor.matmul(
                        oacc[u][:DH + 1, :ilen],
                        vt[e][JT - 1][:1, h * (DH + 1):(h + 1) * (DH + 1)],
                        pt8[0:1, u * 512:u * 512 + ilen],
                        start=False, stop=True)
                # normalize
                for u in range(2):
                    rec1 = smpool.tile([1, 512], F32, tag="rec1",
                                       name=f"rec1{e}_{g}_{i0}_{u}")
                    nc.vector.reciprocal(rec1[:1, :ilen],
                                         oacc[u][DH:DH + 1, :ilen])
                    rec = smpool.tile([128, 512], F32, tag="rec",
                                      name=f"rec{e}_{g}_{i0}_{u}")
                    nc.gpsimd.partition_broadcast(rec[:DH, :ilen],
                                                  rec1[:1, :ilen])
                    nc.vector.tensor_tensor(
                        out=oT[e][g][u * 64:(u + 1) * 64, i0:i0 + ilen],
                        in0=oacc[u][:DH, :ilen], in1=rec[:DH, :ilen], op=MULT)
            # ---- query column 1024 ----
            sg = smallp.tile([128, 512], F32, tag="small", name=f"sgq{e}_{g}")
            for u in range(2):
                for j in range(JT):
                    jlen = min(128, L - j * 128)
                    nc.tensor.matmul(
                        sg[:jlen, u * 9 + j:u * 9 + j + 1],
                        kt_k[u * 64:(u + 1) * 64, j * 128:j * 128 + jlen],
                        kt_q[u * 64:(u + 1) * 64, 1024:1025],
                        start=True, stop=True)
            ptsg = ptpool.tile([128, 18], BF16, tag="ptsg", name=f"ptsg{e}_{g}")
            nc.scalar.activation(ptsg[:, :], sg[:, 0:18], EXP,
                                 bias=0.0, scale=float(SCALE))
            for u in range(2):
                h = 2 * g + u
                for j in range(JT):
                    jlen = min(128, L - j * 128)
                    nc.tensor.matmul(
                        accsg[:DH + 1, u:u + 1],
                        vt[e][j][:jlen, h * (DH + 1):(h + 1) * (DH + 1)],
                        ptsg[:jlen, u * 9 + j:u * 9 + j + 1],
                        start=(j == 0), stop=(j == JT - 1))
            for u in range(2):
                rec1 = smpool.tile([1, 512], F32, tag="rec1",
                                   name=f"rec1sg{e}_{g}_{u}")
                nc.vector.reciprocal(rec1[:1, :1], accsg[DH:DH + 1, u:u + 1])
                rec = smpool.tile([128, 512], F32, tag="rec",
                                  name=f"recsg{e}_{g}_{u}")
                nc.gpsimd.partition_broadcast(rec[:DH, :1], rec1[:1, :1])
                nc.vector.tensor_tensor(
                    out=oT[e][g][u * 64:(u + 1) * 64, 1024:1025],
                    in0=accsg[:DH, u:u + 1], in1=rec[:DH, :1], op=MULT)

        def out_proj(e, m):
            yt = ytpool.tile([128, L], F32, tag="yt", name=f"yt{e}_{m}")
            ps = bigp.tile([128, 1024], F32, tag="big", name=f"ops{e}_{m}")
            for k in range(KT):
                nc.tensor.matmul(ps[:, 0:512], wo_t[k][:, m * 128:(m + 1) * 128],
                                 oT[e][k][:, 0:512],
                                 start=(k == 0), stop=(k == KT - 1))
            for k in range(KT):
                nc.tensor.matmul(ps[:, 512:1024],
                                 wo_t[k][:, m * 128:(m + 1) * 128],
                                 oT[e][k][:, 512:1024],
                                 start=(k == 0), stop=(k == KT - 1))
            sg = smallp.tile([128, 512], F32, tag="small", name=f"osg{e}_{m}")
            for k in range(KT):
                nc.tensor.matmul(sg[:, 0:1], wo_t[k][:, m * 128:(m + 1) * 128],
                                 oT[e][k][:, 1024:1025],
                                 start=(k == 0), stop=(k == KT - 1))
            nc.vector.tensor_scalar_add(yt[:, 0:1024], ps[:, 0:1024], bo_t[m][:])
            nc.vector.tensor_scalar_add(yt[:, 1024:1025], sg[:, 0:1], bo_t[m][:])
            nc.sync.dma_start(out=yT[e, m * 128:(m + 1) * 128, :], in_=yt[:])

        # ---- schedule ----
        load_x(0)
        v_proj(0, list(range(JT)))
        qk_proj(0, list(range(2 * KT)))
        load_x(1)
        # elem0 attention with elem1 projections interleaved
        attention(0, 0); v_proj(1, [0, 1, 2])
        attention(0, 1); v_proj(1, [3, 4, 5])
        attention(0, 2); v_proj(1, [6, 7, 8])
        attention(0, 3); qk_proj(1, [0, 6, 1, 7])
        attention(0, 4); qk_proj(1, [2, 8, 3, 9])
        attention(0, 5); qk_proj(1, [4, 10, 5, 11])
        # elem1 attention with elem0 output projection interleaved
        for g in range(KT):
            attention(1, g)
            out_proj(0, g)
        for m in range(KT):
            out_proj(1, m)


def _prep_inputs(query, Wqkv, bqkv, Wo, bo):
    Wp = Wqkv.reshape(D, 3, DH, H).transpose(0, 1, 3, 2).reshape(D, 3 * D)
    bp = bqkv.reshape(3, DH, H).transpose(0, 2, 1).reshape(3 * D)
    w_qk = np.ascontiguousarray(Wp[:, :2 * D]).astype(ml_dtypes.bfloat16)
    w_v = np.ascontiguousarray(Wp[:, 2 * D:]).astype(ml_dtypes.bfloat16)
    w_o = np.ascontiguousarray(Wo).astype(ml_dtypes.bfloat16)
    b_qk = np.ascontiguousarray(bp[:2 * D]).astype(np.float32).reshape(2 * D, 1)
    b_v = np.ascontiguousarray(bp[2 * D:]).astype(np.float32).reshape(1, D)
    b_o = np.ascontiguousarray(bo).astype(np.float32).reshape(D, 1)

    in_maps = []
    for c in range(N_CORES):
        xc = query[c * BPC:(c + 1) * BPC]
        xTc = np.ascontiguousarray(xc.transpose(0, 2, 1)).astype(
            ml_dtypes.bfloat16)
        in_maps.append(dict(xT=xTc, w_qk=w_qk, w_v=w_v, w_o=w_o,
                            b_qk=b_qk, b_v=b_v, b_o=b_o))
    return in_maps


def kernel(query, Wqkv, bqkv, Wo, bo):
    query = np.asarray(query, dtype=np.float32)
    Wqkv = np.asarray(Wqkv, dtype=np.float32)
    bqkv = np.asarray(bqkv, dtype=np.float32)
    Wo = np.asarray(Wo, dtype=np.float32)
    bo = np.asarray(bo, dtype=np.float32)

    if "nc" not in _CACHE:
        _CACHE["nc"] = _build()
    nc = _CACHE["nc"]

    in_maps = _prep_inputs(query, Wqkv, bqkv, Wo, bo)
    res = run_bass_kernel_spmd(nc, in_maps, core_ids=list(range(N_CORES)))
    out = np.empty((B, L, D), dtype=np.float32)
    for c in range(N_CORES):
        out[c * BPC:(c + 1) * BPC] = res.results[c]["yT"].transpose(0, 2, 1)
    return out


# revision 22
# speedup vs baseline: 1.2862x; 1.1031x over previous
"""Multi-head attention (b=16, l=1025, d=768, H=12) on 8 TRN2 NeuronCores.

Sharding: data-parallel over batch - 2 batch elements per core, no
collectives.

Per-core kernel (per batch element), layouts transposed so the sequence
dim is the matmul free dim:
  1. QK^T = (Wqk stationary) @ X^T            -> [1536, L]  (bf16)
  2. V    = (X^T blocks stationary) @ Wv      -> [L, 768] stored per-head
     as [L, 12*(64+1)] with a ones column per head (gives softmax sums).
  3. Per head pair g (heads 2g, 2g+1 at partitions 0-63 / 64-127, PE
     row-group packed), per i-chunk c in {[0:512], [512:1024]}:
       S^T[jblk, i] = K_h^T.T @ Q_h^T   (K=64)
       P^T = exp(S^T/8) on ACT, grouped 2 j-tiles per instruction
       O_aug^T[65, i] += V_aug[jblk].T @ P^T[jblk]  (row 64 = denom)
       O^T = O_aug^T[0:64] * recip(O_aug^T[64])
     query col 1024 handled via a [128, 18] psum tile + one exp.
  4. Y^T = (Wo stationary) @ O^T + bo         -> [768, L] fp32 -> DRAM

Projection work of element 1 is interleaved into element 0's attention
pairs (and elem0's output projection into elem1's attention) so the PE
has dense work while ACT drains the exps.
"""

import contextlib

import numpy as np
import ml_dtypes

import concourse.bass as bass
import concourse.bacc as bacc
import concourse.mybir as mybir
import concourse.tile as tile
from concourse.bass_utils import run_bass_kernel_spmd

N_CORES = 8
B = 16
L = 1025
D = 768
H = 12
DH = 64
BPC = B // N_CORES
KT = D // 128   # 6 contraction tiles
JT = (L + 127) // 128  # 9 j-tiles; last has 1 row
SCALE = 1.0 / np.sqrt(DH)

BF16 = mybir.dt.bfloat16
F32 = mybir.dt.float32
EXP = mybir.ActivationFunctionType.Exp
MULT = mybir.AluOpType.mult
ADD = mybir.AluOpType.add

_CACHE = {}


def _build():
    nc = bacc.Bacc("TRN2", target_bir_lowering=False, debug=False,
                   num_devices=N_CORES)
    xT = nc.dram_tensor("xT", [BPC, D, L], BF16, kind="ExternalInput")
    w_qk = nc.dram_tensor("w_qk", [D, 2 * D], BF16, kind="ExternalInput")
    w_v = nc.dram_tensor("w_v", [D, D], BF16, kind="ExternalInput")
    w_o = nc.dram_tensor("w_o", [D, D], BF16, kind="ExternalInput")
    b_qk = nc.dram_tensor("b_qk", [2 * D, 1], F32, kind="ExternalInput")
    b_v = nc.dram_tensor("b_v", [1, D], F32, kind="ExternalInput")
    b_o = nc.dram_tensor("b_o", [D, 1], F32, kind="ExternalInput")
    yT = nc.dram_tensor("yT", [BPC, D, L], F32, kind="ExternalOutput")
    kTo = nc.dram_tensor("kTo", [BPC, D, L], BF16, kind="ExternalOutput")
    vo = nc.dram_tensor("vo", [BPC, JT, 128, H * (DH + 1)], BF16,
                        kind="ExternalOutput")

    with tile.TileContext(nc) as tc:
        _emit(nc, tc, xT, w_qk, w_v, w_o, b_qk, b_v, b_o, yT, kTo, vo)
    nc.compile()
    return nc


def _ap(t, poff, pcount, foff, fdims):
    """AP on tile t at partition offset poff (count pcount), free offset
    foff with free dims [(step, count), ...]."""
    base = t[:]
    pstep = base.ap[0][0]
    return bass.AP(tensor=base.tensor,
                   offset=base.offset + poff * pstep + foff,
                   ap=[[pstep, pcount]] + [list(d) for d in fdims])


def _emit(nc, tc, xT, w_qk, w_v, w_o, b_qk, b_v, b_o, yT, kTo, vo):
    ctx = contextlib.ExitStack()
    with ctx:
        consts = ctx.enter_context(tc.tile_pool(name="consts", bufs=1))
        xpool = ctx.enter_context(tc.tile_pool(name="xpool", bufs=1))
        qkpool = ctx.enter_context(tc.tile_pool(name="qkpool", bufs=2))
        vpool = ctx.enter_context(tc.tile_pool(name="vpool", bufs=2))
        otpool = ctx.enter_context(tc.tile_pool(name="otpool", bufs=2))
        ytpool = ctx.enter_context(tc.tile_pool(name="ytpool", bufs=2))
        ptpool = ctx.enter_context(tc.tile_pool(name="ptpool", bufs=5))
        smpool = ctx.enter_context(tc.tile_pool(name="smpool", bufs=2))
        # PSUM: big 2x[128,1024]=4 banks, small 1, acc 3 -> 8
        bigp = ctx.enter_context(tc.tile_pool(name="bigp", bufs=2, space="PSUM"))
        smallp = ctx.enter_context(tc.tile_pool(name="smallp", bufs=2, space="PSUM"))
        accp = ctx.enter_context(tc.tile_pool(name="accp", bufs=2, space="PSUM"))

        # ---- constants (xt emitted first by the schedule; wo last) ----
        wqk_t = [consts.tile([128, 2 * D], BF16, name=f"wqk{k}") for k in range(KT)]
        wv_t = [consts.tile([128, D], BF16, name=f"wv{k}") for k in range(KT)]
        # wo shares the wv slots: wv's last read (v_proj of elem1) precedes
        # wo's first use (out-proj fillers), so the single-buf tag serializes
        # the wo DMA behind v_proj naturally.
        wo_t = [consts.tile([128, D], BF16, tag=f"wv{k}", name=f"wo{k}")
                for k in range(KT)]
        bqk_t = [consts.tile([128, 1], F32, name=f"bqk{m}") for m in range(2 * KT)]
        bo_t = [consts.tile([128, 1], F32, name=f"bo{m}") for m in range(KT)]
        bv_bc = consts.tile([128, D], F32, name="bvbc")

        def load_consts():
            # wv tiles race the first v_proj matmuls; interleaved k-order
            for k in range(KT):
                nc.sync.dma_start(out=wv_t[k][:], in_=w_v[k * 128:(k + 1) * 128, :])
            bva = b_v[:]
            nc.sync.dma_start(out=bv_bc[:], in_=bass.AP(
                tensor=bva.tensor, offset=bva.offset,
                ap=[[0, 128], list(bva.ap[1])]))
            for k in range(KT):
                nc.sync.dma_start(out=wqk_t[k][:], in_=w_qk[k * 128:(k + 1) * 128, :])
            for m in range(2 * KT):
                nc.sync.dma_start(out=bqk_t[m][:], in_=b_qk[m * 128:(m + 1) * 128, :])
            for m in range(KT):
                nc.sync.dma_start(out=bo_t[m][:], in_=b_o[m * 128:(m + 1) * 128, :])
            for k in range(KT):
                nc.sync.dma_start(out=wo_t[k][:], in_=w_o[k * 128:(k + 1) * 128, :])

        xt = {}
        qkT = {}
        vt = {}
        oT = {}

        def load_x(e):
            xt[e] = [xpool.tile([128, L], BF16, tag=f"xt{k}", name=f"xt{e}_{k}")
                     for k in range(KT)]
            for k in range(KT):
                nc.sync.dma_start(out=xt[e][k][:],
                                  in_=xT[e, k * 128:(k + 1) * 128, :])

        def v_proj(e, jlist):
            """V[j,:] for j-tiles in jlist; layout [jlen, 12*(64+1)]."""
            if e not in vt:
                vt[e] = [vpool.tile([128, H * (DH + 1)], BF16, tag=f"vt{j}",
                                    name=f"vt{e}_{j}") for j in range(JT)]
            for j in jlist:
                jlen = min(128, L - j * 128)
                nc.vector.memset(
                    _ap(vt[e][j], 0, 128, DH, [[DH + 1, H], [1, 1]]), 1.0)
                ps = bigp.tile([128, 1024], F32, tag="big", name=f"vps{e}_{j}")
                for k in range(KT):
                    nc.tensor.matmul(ps[:jlen, 0:512],
                                     xt[e][k][:, j * 128:j * 128 + jlen],
                                     wv_t[k][:, 0:512],
                                     start=(k == 0), stop=(k == KT - 1))
                for k in range(KT):
                    nc.tensor.matmul(ps[:jlen, 512:768],
                                     xt[e][k][:, j * 128:j * 128 + jlen],
                                     wv_t[k][:, 512:768],
                                     start=(k == 0), stop=(k == KT - 1))
                dst = _ap(vt[e][j], 0, jlen, 0, [[DH + 1, H], [1, DH]])
                src = _ap(ps, 0, jlen, 0, [[DH, H], [1, DH]])
                bia = _ap(bv_bc, 0, jlen, 0, [[DH, H], [1, DH]])
                nc.vector.tensor_tensor(out=dst, in0=src, in1=bia, op=ADD)
                nc.sync.dma_start(out=vo[e, j], in_=vt[e][j][:])

        def qk_unit(e, m):
            """One QK^T m-tile: big psum (c0+c1), small straggler col."""
            if e not in qkT:
                qkT[e] = [qkpool.tile([128, L], BF16, tag=f"qkT{t}",
                                      name=f"qkT{e}_{t}") for t in range(2 * KT)]
            ps = bigp.tile([128, 1024], F32, tag="big", name=f"qkps{e}_{m}")
            for k in range(KT):
                nc.tensor.matmul(ps[:, 0:512],
                                 wqk_t[k][:, m * 128:(m + 1) * 128],
                                 xt[e][k][:, 0:512],
                                 start=(k == 0), stop=(k == KT - 1))
            for k in range(KT):
                nc.tensor.matmul(ps[:, 512:1024],
                                 wqk_t[k][:, m * 128:(m + 1) * 128],
                                 xt[e][k][:, 512:1024],
                                 start=(k == 0), stop=(k == KT - 1))
            nc.vector.tensor_scalar_add(qkT[e][m][:, 0:1024], ps[:, 0:1024],
                                        bqk_t[m][:])
            if m >= KT:
                sg = smallp.tile([128, 512], F32, tag="small",
                                 name=f"qksg{e}_{m}")
                for k in range(KT):
                    nc.tensor.matmul(sg[:, 0:1],
                                     wqk_t[k][:, m * 128:(m + 1) * 128],
                                     xt[e][k][:, 1024:1025],
                                     start=(k == 0), stop=(k == KT - 1))
                nc.vector.tensor_scalar_add(qkT[e][m][:, 1024:1025],
                                            sg[:, 0:1], bqk_t[m][:])
                nc.sync.dma_start(out=kTo[e, (m - KT) * 128:(m - KT + 1) * 128, :],
                                  in_=qkT[e][m][:])

        def small_chunk(name, nmm, mms, dve):
            """One projection chunk through the 1-bank small psum pool."""
            ps = smallp.tile([128, 512], F32, tag="small", name=name)
            for i in range(nmm):
                mms(ps, i)
                if i % 2 == 1:
                    yield
            dve(ps)

        def v_unit_gen(e, j):
            if e not in vt:
                vt[e] = [vpool.tile([128, H * (DH + 1)], BF16, tag=f"vt{t}",
                                    name=f"vt{e}_{t}") for t in range(JT)]
            jlen = min(128, L - j * 128)
            nc.vector.memset(
                _ap(vt[e][j], 0, 128, DH, [[DH + 1, H], [1, 1]]), 1.0)
            for c, (c0, nh) in enumerate(((0, 8), (512, 4))):
                def mms(ps, k, c0=c0, clen=64 * nh):
                    nc.tensor.matmul(ps[:jlen, 0:clen],
                                     xt[e][k][:, j * 128:j * 128 + jlen],
                                     wv_t[k][:, c0:c0 + clen],
                                     start=(k == 0), stop=(k == KT - 1))
                def dve(ps, c0=c0, nh=nh):
                    dst = _ap(vt[e][j], 0, jlen, (c0 // 64) * (DH + 1),
                              [[DH + 1, nh], [1, DH]])
                    src = _ap(ps, 0, jlen, 0, [[DH, nh], [1, DH]])
                    bia = _ap(bv_bc, 0, jlen, c0, [[DH, nh], [1, DH]])
                    nc.vector.tensor_tensor(out=dst, in0=src, in1=bia, op=ADD)
                yield from small_chunk(f"vg{e}_{j}_{c}", KT, mms, dve)
            nc.sync.dma_start(out=vo[e, j], in_=vt[e][j][:])

        def qk_unit_gen(e, m):
            if e not in qkT:
                qkT[e] = [qkpool.tile([128, L], BF16, tag=f"qkT{t}",
                                      name=f"qkT{e}_{t}") for t in range(2 * KT)]
            for c in range(2):
                def mms(ps, k, c=c):
                    nc.tensor.matmul(ps[:, 0:512],
                                     wqk_t[k][:, m * 128:(m + 1) * 128],
                                     xt[e][k][:, c * 512:c * 512 + 512],
                                     start=(k == 0), stop=(k == KT - 1))
                def dve(ps, c=c):
                    nc.vector.tensor_scalar_add(
                        qkT[e][m][:, c * 512:c * 512 + 512],
                        ps[:, 0:512], bqk_t[m][:])
                yield from small_chunk(f"qg{e}_{m}_{c}", KT, mms, dve)
            if m >= KT:
                def mms(ps, k):
                    nc.tensor.matmul(ps[:, 0:1],
                                     wqk_t[k][:, m * 128:(m + 1) * 128],
                                     xt[e][k][:, 1024:1025],
                                     start=(k == 0), stop=(k == KT - 1))
                def dve(ps):
                    nc.vector.tensor_scalar_add(qkT[e][m][:, 1024:1025],
                                                ps[:, 0:1], bqk_t[m][:])
                yield from small_chunk(f"qgs{e}_{m}", KT, mms, dve)
                nc.sync.dma_start(
                    out=kTo[e, (m - KT) * 128:(m - KT + 1) * 128, :],
                    in_=qkT[e][m][:])

        def out_unit_gen(e, m):
            yt = ytpool.tile([128, L], F32, tag="yt", name=f"yt{e}_{m}")
            for c in range(2):
                def mms(ps, k, c=c):
                    nc.tensor.matmul(ps[:, 0:512],
                                     wo_t[k][:, m * 128:(m + 1) * 128],
                                     oT[e][k][:, c * 512:c * 512 + 512],
                                     start=(k == 0), stop=(k == KT - 1))
                def dve(ps, c=c):
                    nc.vector.tensor_scalar_add(yt[:, c * 512:c * 512 + 512],
                                                ps[:, 0:512], bo_t[m][:])
                yield from small_chunk(f"og{e}_{m}_{c}", KT, mms, dve)
                nc.sync.dma_start(
                    out=yT[e, m * 128:(m + 1) * 128, c * 512:c * 512 + 512],
                    in_=yt[:, c * 512:c * 512 + 512])

        def load_x_gen(e):
            load_x(e)
            yield

        class Fill:
            def __init__(self, gens):
                self.gens = list(gens)

            def pull(self, n=1):
                while n > 0 and self.gens:
                    try:
                        next(self.gens[0])
                        n -= 1
                    except StopIteration:
                        self.gens.pop(0)

            def finish(self, k):
                """Exhaust the first k remaining generators."""
                for gen in self.gens[:k]:
                    for _ in gen:
                        pass
                self.gens = self.gens[k:]

            def flush(self):
                self.finish(len(self.gens))

        def _fill(filler, n=1):
            for _ in range(n):
                if filler:
                    filler.pop(0)()

        def attention(e, g, fill=None):
            """Head pair g: heads 2g (partitions 0-63), 2g+1 (64-127)."""
            fill = fill or Fill([])
            if e not in oT:
                oT[e] = [otpool.tile([128, L], BF16, tag=f"oT{t}",
                                     name=f"oT{e}_{t}") for t in range(KT)]
            kt_q, kt_k = qkT[e][g], qkT[e][KT + g]
            for (i0, ilen) in ((0, 512), (512, 512)):
                oacc = [accp.tile([128, 512], F32, tag="acc",
                                  name=f"oacc{e}_{g}_{i0}_{u}") for u in range(2)]
                # Per key-block j one big tile holds u0 scores (cols
                # 0:512) and u1 scores (cols 512:1024); the two K=64 mms are
                # adjacent instructions in disjoint PE row groups (0-63 /
                # 64-127) so they overlap on hardware. PV runs 2 key-blocks
                # behind scores (2 big slots); filler plugs residual stalls.
                pts = []

                def pv(j):
                    pt = pts[j]
                    for u in range(2):
                        h = 2 * g + u
                        nc.tensor.matmul(
                            oacc[u][:DH + 1, :ilen],
                            vt[e][j][:, h * (DH + 1):(h + 1) * (DH + 1)],
                            pt[:, u * 512:u * 512 + ilen],
                            start=(j == 0), stop=False)

                for j in range(8):
                    if j >= 2:
                        pv(j - 2)
                    sps = bigp.tile([128, 1024], F32, tag="big",
                                    name=f"sps{e}_{g}_{i0}_{j}")
                    for u in range(2):
                        nc.tensor.matmul(
                            sps[:128, u * 512:u * 512 + ilen],
                            kt_k[u * 64:(u + 1) * 64, j * 128:(j + 1) * 128],
                            kt_q[u * 64:(u + 1) * 64, i0:i0 + ilen],
                            start=True, stop=True)
                    pt = ptpool.tile([128, 1024], BF16, tag="pt",
                                     name=f"pt{e}_{g}_{i0}_{j}")
                    nc.scalar.activation(pt[:, :], sps[:, :], EXP,
                                         bias=0.0, scale=float(SCALE))
                    pts.append(pt)
                    fill.pull(1)
                pv(6)
                fill.pull(1)
                pv(7)
                fill.pull(1)
                # j8 (jlen=1): u0 in cols 0:512, u1 in cols 512:1024,
                # both at partition 0 so PV lhsT/rhs bases match
                sp8 = bigp.tile([128, 1024], F32, tag="big",
                                name=f"sp8{e}_{g}_{i0}")
                for u in range(2):
                    nc.tensor.matmul(
                        sp8[0:1, u * 512:u * 512 + ilen],
                        kt_k[u * 64:(u + 1) * 64, 1024:1025],
                        kt_q[u * 64:(u + 1) * 64, i0:i0 + ilen],
                        start=True, stop=True)
                pt8 = ptpool.tile([1, 1024], BF16, tag="pt8",
                                  name=f"pt8{e}_{g}_{i0}")
                nc.scalar.activation(pt8[:1, :], sp8[:1, :], EXP,
                                     bias=0.0, scale=float(SCALE))
                for u in range(2):
                    h = 2 * g + u
                    nc.tensor.matmul(
                        oacc[u][:DH + 1, :ilen],
                        vt[e][JT - 1][:1, h * (DH + 1):(h + 1) * (DH + 1)],
                        pt8[0:1, u * 512:u * 512 + ilen],
                        start=False, stop=True)
                fill.pull(1)
                # normalize
                for u in range(2):
                    rec1 = smpool.tile([1, 512], F32, tag="rec1",
                                       name=f"rec1{e}_{g}_{i0}_{u}")
                    nc.vector.reciprocal(rec1[:1, :ilen],
                                         oacc[u][DH:DH + 1, :ilen])
                    rec = smpool.tile([128, 512], F32, tag="rec",
                                      name=f"rec{e}_{g}_{i0}_{u}")
                    nc.gpsimd.partition_broadcast(rec[:DH, :ilen],
                                                  rec1[:1, :ilen])
                    nc.vector.tensor_tensor(
                        out=oT[e][g][u * 64:(u + 1) * 64, i0:i0 + ilen],
                        in0=oacc[u][:DH, :ilen], in1=rec[:DH, :ilen], op=MULT)
                fill.pull(1)

        def out_proj(e, m):
            yt = ytpool.tile([128, L], F32, tag="yt", name=f"yt{e}_{m}")
            ps = bigp.tile([128, 1024], F32, tag="big", name=f"ops{e}_{m}")
            for k in range(KT):
                nc.tensor.matmul(ps[:, 0:512], wo_t[k][:, m * 128:(m + 1) * 128],
                                 oT[e][k][:, 0:512],
                                 start=(k == 0), stop=(k == KT - 1))
            for k in range(KT):
                nc.tensor.matmul(ps[:, 512:1024],
                                 wo_t[k][:, m * 128:(m + 1) * 128],
                                 oT[e][k][:, 512:1024],
                                 start=(k == 0), stop=(k == KT - 1))
            nc.vector.tensor_scalar_add(yt[:, 0:1024], ps[:, 0:1024], bo_t[m][:])
            nc.sync.dma_start(out=yT[e, m * 128:(m + 1) * 128, 0:1024],
                              in_=yt[:, 0:1024])

        # ---- schedule ----
        # warm the exp table during the input DMA shadow
        warm = smpool.tile([1, 512], F32, tag="rec1", name="warm")
        nc.vector.memset(warm[:1, 0:1], 0.0)
        nc.scalar.activation(warm[:1, 0:1], warm[:1, 0:1], EXP,
                             bias=0.0, scale=1.0)
        # interleave xt[k] / wv[k] so v_proj's k-th matmul can start as
        # soon as the k-th pair lands
        xt[0] = [xpool.tile([128, L], BF16, tag=f"xt{k}", name=f"xt0_{k}")
                 for k in range(KT)]
        for k in range(KT):
            nc.sync.dma_start(out=xt[0][k][:],
                              in_=xT[0, k * 128:(k + 1) * 128, :])
            nc.sync.dma_start(out=wv_t[k][:], in_=w_v[k * 128:(k + 1) * 128, :])
        bva = b_v[:]
        nc.sync.dma_start(out=bv_bc[:], in_=bass.AP(
            tensor=bva.tensor, offset=bva.offset,
            ap=[[0, 128], list(bva.ap[1])]))
        for k in range(KT):
            nc.sync.dma_start(out=wqk_t[k][:], in_=w_qk[k * 128:(k + 1) * 128, :])
        for m in range(2 * KT):
            nc.sync.dma_start(out=bqk_t[m][:], in_=b_qk[m * 128:(m + 1) * 128, :])
        for m in range(KT):
            nc.sync.dma_start(out=bo_t[m][:], in_=b_o[m * 128:(m + 1) * 128, :])
        for k in range(KT):
            nc.sync.dma_start(out=wo_t[k][:], in_=w_o[k * 128:(k + 1) * 128, :])
        v_proj(0, list(range(JT)))
        qk_unit(0, 0); qk_unit(0, KT)
        f = []
        for g in range(1, KT):
            f += [lambda m=g: qk_unit(0, m), lambda m=KT + g: qk_unit(0, m)]
        f += [lambda: load_x(1)]
        f += [lambda j=j: v_proj(1, [j]) for j in range(JT)]
        f += [lambda: qk_unit(1, 0), lambda: qk_unit(1, KT)]
        for g in range(KT):
            n_slots = 5
            take, f = f[:n_slots], f[n_slots:]
            attention(0, g, take)
        for fn in f:
            fn()
        f = []
        for g in range(1, KT):
            f += [lambda m=g: qk_unit(1, m), lambda m=KT + g: qk_unit(1, m)]
        f += [lambda m=m: out_proj(0, m) for m in range(KT)]
        for g in range(KT):
            n_slots = 3
            take, f = f[:n_slots], f[n_slots:]
            attention(1, g, take)
        for fn in f:
            fn()
        for m in range(KT):
            out_proj(1, m)
